# revision 5
# baseline (speedup 1.0000x reference)
"""Fused MoE (top-2 routing) on 8 trn2 NeuronCores, expert-parallel.

Strategy: E=16 experts are sharded 2-per-core (two "slots"). The host groups
the T*TOPK (token, slot) pairs by expert (the all-to-all "dispatch"), assigns
the 8 largest experts to slot 0 and the 8 smallest to slot 1, and sizes the
two slot capacities (c0 >= c1) at runtime from the actual routing counts —
the kernel is rebuilt (and NEFF-cached) per capacity pair, so no compute or
DMA is spent on empty padding beyond round-to-2.

All matmul inputs and the y output travel as bf16 (fp32 accumulate in PSUM);
rel-err vs the f32 reference is ~5e-3, inside the 2e-2 gate, and DMA bytes
halve vs f32/f32r. Per core, per slot with capacity c:
  - xT_s  [128, 8*c]   gathered tokens, row p col kc*c+j = x[tok j, k=kc*128+p]
  - wup_s [128, 8*512] up_weight[e].T in the same [p, kc, f] layout
  - wdn_s [128, 2*1024] down_weight[e].T, row p col hh*1024+k
  - wv    [128, nd0+nd1] routing weight per pair, [p, token-chunk] layout
Every load row is a single 4-9KB contiguous DRAM run per partition: DMA here
is descriptor-rate-bound (~80ns per descriptor per SDMA queue), so loads are
merged into halves (x, wup) or whole tensors (wdn) — 128 descriptors of
4-9KB each instead of 256+ of 1-2KB.

The device computes up.T = wupT.T @ xT (PSUM fp32), SwiGLU in the transposed
layout (no on-chip transposes), down = actT.T @ wdnT with the routing weight
applied on the PSUM->SBUF copy, and writes y_s [c, K] bf16. Phases run
up(0), down(0), up(1), down(1) so slot-0 stores stream mid-kernel. The host
scatter-adds y rows back to tokens in f32 (the "combine").

ALL DMA (loads in consumption order, then stores) goes on the sync-engine
HWDGE queue: it fans out across all 16 SDMA engines, while the scalar-engine
ring only drives ~2 queues (v2 measured: scalar-queue stores trickled 8us
past the last matmul). The leading wait-free load triggers are hoisted into
the NEFF entry block so transfers stream during the preamble, a short run of
wait-free dummy matmuls (on garbage SBUF) is hoisted likewise so the PE HAM
clock-gate is already at 2.4GHz when the first real matmul issues, and
Tile's redundant exit semaphore-clear/barrier is stripped.
"""

import os

import ml_dtypes
import numpy as np

import concourse.bass as bass
import concourse.mybir as mybir
from concourse.bass_utils import run_bass_kernel_spmd
from concourse.tile import TileContext

T, K, H, E, TOPK = 4096, 1024, 256, 16, 2
H2 = 2 * H  # 512
NCORES = 8
EPC = E // NCORES  # experts (slots) per core = 2
KC = K // 128  # 8 contraction chunks
CAPMAX = 2048  # per-slot capacity bound (SBUF); above this, run more rounds

F32 = mybir.dt.float32
# matmul input dtype: "bf16" (half DMA bytes) or "f32r" (tf32-like, full bytes)
MM_DTYPE = os.environ.get("MOE_MM_DTYPE", "bf16")
Y_DTYPE = os.environ.get("MOE_Y_DTYPE", MM_DTYPE)
NHOIST = int(os.environ.get("MOE_NHOIST", "16"))
NWARM = int(os.environ.get("MOE_WARM", "12"))
WARM_HOIST = os.environ.get("MOE_WARM_HOIST", "1") == "1"


def _up_tiles(c):
    """Split c token columns into PSUM-bank-sized (<=512) even chunks."""
    if c <= 512:
        return [c]
    u = (c + 3) // 4 * 2
    return [u, c - u]


def _fix_multi_waits(nc):
    """This walrus build accepts one sync-wait command per instruction (two
    for EventSemaphore); Tile's exit drain stacks every outstanding semaphore
    onto a single Drain. Move the excess waits onto no-ops inserted before
    the offending instruction on the same engine."""
    for f in nc.m.functions:
        for bb in f.blocks:
            i = 0
            while i < len(bb.instructions):
                ins = bb.instructions[i]
                si = ins.sync_info
                cap = 2 if isinstance(ins, mybir.InstEventSemaphore) else 1
                if si is not None and si.on_wait and len(si.on_wait) > cap:
                    waits = list(si.on_wait)
                    keep, extra = waits[:cap], waits[cap:]
                    nops = [
                        mybir.InstNoOp(
                            name=f"{ins.name}_waitfix{j}",
                            sync_info=mybir.SyncInfo(on_wait=[w], on_update=[]),
                            bass_nofuse=True,
                            engine=ins.engine,
                        )
                        for j, w in enumerate(extra)
                    ]
                    ins.sync_info = mybir.SyncInfo(
                        on_wait=keep, on_update=list(si.on_update)
                    )
                    bb.instructions[i:i] = nops
                    i += len(nops)
                i += 1


_NC_CACHE = {}


def _build(caps):
    key = (tuple(caps), MM_DTYPE, Y_DTYPE, NWARM, WARM_HOIST)
    if key in _NC_CACHE:
        return _NC_CACHE[key]
    DT = mybir.dt.float32r if MM_DTYPE == "f32r" else mybir.dt.bfloat16
    YDT = mybir.dt.float32 if Y_DTYPE == "f32r" else mybir.dt.bfloat16
    nds = [-(-c // 128) for c in caps]  # down-phase token tiles per slot
    nc = bass.Bass()
    xT = [
        nc.dram_tensor(f"xT{s}", [128, KC * caps[s]], DT, kind="ExternalInput")
        for s in range(EPC)
    ]
    wup = [
        nc.dram_tensor(f"wup{s}", [128, KC * H2], DT, kind="ExternalInput")
        for s in range(EPC)
    ]
    wdn = [
        nc.dram_tensor(f"wdn{s}", [128, 2 * K], DT, kind="ExternalInput")
        for s in range(EPC)
    ]
    wv = nc.dram_tensor("wv", [128, sum(nds)], F32, kind="ExternalInput")
    y = [
        nc.dram_tensor(f"y{s}", [caps[s], K], YDT, kind="ExternalOutput")
        for s in range(EPC)
    ]

    with TileContext(nc) as tc:
        with (
            tc.tile_pool(name="persist", bufs=1) as pp,
            tc.tile_pool(name="sil", bufs=4) as silp,
            tc.tile_pool(name="yout", bufs=6) as yp,
            tc.tile_pool(name="psum_up", bufs=2, space="PSUM") as psu,
            tc.tile_pool(name="psum_dn", bufs=2, space="PSUM") as psd,
        ):
            # PE warm-up: wait-free matmuls on never-written SBUF, hoisted
            # into the entry block so the HAM clock-gate sees ~3.4us of
            # sustained PE activity during the preamble and releases the
            # 1.2GHz->2.4GHz throttle before the first real matmul.
            if NWARM:
                wtile = pp.tile([128, 512], DT, tag="warm", name="wtile")
                nc.vector.memset(wtile[:], 0.0)
                pwarm = psd.tile([128, 512], F32, tag="dn0", name="pwarm")
                for _ in range(NWARM):
                    nc.tensor.matmul(
                        pwarm[:], wtile[:, :128], wtile[:], start=True, stop=True
                    )

            # one tile per (tensor, slot, half) so readers only gate on the
            # DMA that actually feeds them; each DMA's DRAM rows are one
            # contiguous 4-9KB run per partition
            xsb = [
                [
                    pp.tile(
                        [128, 4, caps[s]], DT, tag=f"x{s}_{h}", name=f"x{s}_{h}"
                    )
                    for h in range(2)
                ]
                for s in range(EPC)
            ]
            wupsb = [
                [
                    pp.tile(
                        [128, 4, H2], DT, tag=f"wu{s}_{h}", name=f"wu{s}_{h}"
                    )
                    for h in range(2)
                ]
                for s in range(EPC)
            ]
            wdnsb = [
                pp.tile([128, 2, K], DT, tag=f"wd{s}", name=f"wd{s}")
                for s in range(EPC)
            ]
            actsb = [
                [
                    pp.tile(
                        [128, caps[s]], DT, tag=f"a{s}_{hh}", name=f"a{s}_{hh}"
                    )
                    for hh in range(2)
                ]
                for s in range(EPC)
            ]
            wvsb = pp.tile([128, sum(nds)], F32)

            # all loads on the sync HWDGE queue, in consumption order
            def load_wup(s, h):
                nc.sync.dma_start(
                    wupsb[s][h][:],
                    wup[s][
                        :, h * 4 * H2 : (h + 1) * 4 * H2
                    ].rearrange("p (kc f) -> p kc f", kc=4),
                )

            def load_x(s, h):
                c = caps[s]
                nc.sync.dma_start(
                    xsb[s][h][:],
                    xT[s][
                        :, h * 4 * c : (h + 1) * 4 * c
                    ].rearrange("p (kc j) -> p kc j", kc=4),
                )

            def load_wdn(s):
                nc.sync.dma_start(
                    wdnsb[s][:],
                    wdn[s][:, :].rearrange("p (hh k) -> p hh k", hh=2),
                )

            for h in range(2):
                load_wup(0, h)
                load_x(0, h)
            load_wdn(0)
            nc.sync.dma_start(wvsb[:], wv[:, :])
            for h in range(2):
                load_wup(1, h)
                load_x(1, h)
            load_wdn(1)

            def up_phase(s):
                # up.T in PSUM: [feature-on-partition, token-free]. Features
                # hh*128..hh*128+127 (gate) pair with 256+hh*128.. (proj);
                # process one hh-half at a time so only two PSUM tags are
                # live and halves pipeline through 2 bufs each.
                c0 = 0
                for ulen in _up_tiles(caps[s]):
                    for hh in range(2):
                        pg = psu.tile([128, 512], F32, tag="upA", name="pg")[
                            :, :ulen
                        ]
                        pj = psu.tile([128, 512], F32, tag="upB", name="pj")[
                            :, :ulen
                        ]
                        for kc in range(KC):
                            rhs = xsb[s][kc // 4][:, kc % 4, c0 : c0 + ulen]
                            nc.tensor.matmul(
                                pg,
                                wupsb[s][kc // 4][
                                    :, kc % 4, hh * 128 : (hh + 1) * 128
                                ],
                                rhs,
                                start=(kc == 0),
                                stop=(kc == KC - 1),
                            )
                            nc.tensor.matmul(
                                pj,
                                wupsb[s][kc // 4][
                                    :, kc % 4, 256 + hh * 128 : 384 + hh * 128
                                ],
                                rhs,
                                start=(kc == 0),
                                stop=(kc == KC - 1),
                            )
                        sil = silp.tile([128, 512], F32, tag="sil")
                        nc.scalar.activation(
                            sil[:, :ulen], pg, mybir.ActivationFunctionType.Silu
                        )
                        nc.vector.tensor_tensor(
                            actsb[s][hh][:, c0 : c0 + ulen],
                            sil[:, :ulen],
                            pj,
                            mybir.AluOpType.mult,
                        )
                    c0 += ulen

            def down_phase(s):
                # down: [token-on-partition, k-free]; routing weight applied
                # on the PSUM->SBUF copy (split across DVE and ACT); stores
                # follow the loads on the sync queue (the only ring that
                # fans out across all 16 SDMA engines)
                base = s * nds[0]
                for td in range(nds[s]):
                    pt = min(128, caps[s] - td * 128)
                    ysb = yp.tile([128, K], YDT, tag="y", name="ysb")
                    col = base + td
                    wcol = wvsb[:pt, col : col + 1]
                    pys = [
                        psd.tile([128, 512], F32, tag=f"dn{nn}", name=f"dn{nn}")
                        for nn in range(2)
                    ]
                    # scale of the first half runs while the second half's
                    # matmuls stream, shortening the per-block PSUM recycle
                    # and the end-of-kernel chain
                    for nn in range(2):
                        for hh in range(2):
                            nc.tensor.matmul(
                                pys[nn][:pt],
                                actsb[s][hh][:, td * 128 : td * 128 + pt],
                                wdnsb[s][:, hh, nn * 512 : (nn + 1) * 512],
                                start=(hh == 0),
                                stop=(hh == 1),
                            )
                        if nn == 0:
                            nc.vector.tensor_scalar_mul(
                                ysb[:pt, 0:512], pys[0][:pt], wcol
                            )
                    nc.scalar.mul(ysb[:pt, 512:1024], pys[1][:pt], wcol)
                    r0 = td * 128
                    nc.sync.dma_start(y[s][r0 : r0 + pt, :], ysb[:pt])

            up_phase(0)
            down_phase(0)
            up_phase(1)
            down_phase(1)

    # Hoist the leading wait-free sync-engine DMA triggers (the loads) into
    # the entry block, ahead of the Tile entry barrier: the transfers then
    # stream during the preamble (IRAM loads, const memsets, barrier)
    # instead of after it. HWDGE triggers retire at descriptor push, so the
    # preamble barrier's Drain does not stall on the in-flight transfers;
    # the body's existing semaphore waits gate consumers exactly as before.
    f0 = nc.m.functions[0]
    blocks = list(f0.blocks)
    main_bb, body_bb = blocks[0], blocks[1]
    hoist = []
    for ins in body_bb.instructions:
        if (
            isinstance(ins, mybir.InstDMACopy)
            and str(ins.engine) == "EngineType.SP"
            and not (ins.sync_info and ins.sync_info.on_wait)
        ):
            hoist.append(ins)
            if len(hoist) >= NHOIST:
                break
        elif isinstance(ins, mybir.InstDMACopy) and str(ins.engine) == (
            "EngineType.SP"
        ):
            break
    if hoist:
        names = {h.name for h in hoist}
        body_bb.instructions[:] = [
            i for i in body_bb.instructions if i.name not in names
        ]
        ip = 0
        for idx, ins in enumerate(main_bb.instructions):
            if str(ins.engine) == "EngineType.SP":
                si = ins.sync_info
                if si and (si.on_wait or si.on_update):
                    break
                ip = idx + 1
        main_bb.instructions[ip:ip] = hoist

    if NWARM and WARM_HOIST:
        # Move the warm-up memset (DVE, wait-free) and the warm-up
        # LDWEIGHTS/Matmults (PE; their only wait is the memset's semaphore,
        # which fires during the preamble) to the entry block, just before
        # each engine's barrier Drain: they then run during the preamble
        # while the PE would otherwise sit idle. Their semaphore updates
        # persist across the entry barrier — nothing in the entry block
        # resets semaphores.
        wh = []
        for ins in body_bb.instructions:
            if (
                isinstance(ins, mybir.InstMemset)
                and str(ins.engine) == "EngineType.DVE"
                and not (ins.sync_info and ins.sync_info.on_wait)
            ):
                wh.append(ins)
                break
        nmm = 0
        for ins in body_bb.instructions:
            if str(ins.engine) != "EngineType.PE":
                continue
            wh.append(ins)
            if isinstance(ins, mybir.InstMatmult):
                nmm += 1
                if nmm >= NWARM:
                    break
        if nmm == NWARM:
            names = {h.name for h in wh}
            body_bb.instructions[:] = [
                i for i in body_bb.instructions if i.name not in names
            ]
            for eng in ("EngineType.DVE", "EngineType.PE"):
                grp = [h for h in wh if str(h.engine) == eng]
                if not grp:
                    continue
                ip = next(
                    idx
                    for idx, ins in enumerate(main_bb.instructions)
                    if isinstance(ins, mybir.InstDrain)
                    and str(ins.engine) == eng
                )
                main_bb.instructions[ip:ip] = grp

    if True:  # drop Tile's exit sem-clear + second barrier (redundant with
        # the compiler's own per-engine semaphore-reset epilogue; verified
        # correct across repeated executions of the loaded NEFF)
        f = nc.m.functions[0]
        endbb = list(f.blocks)[-1]
        # keep: waitfix nops + SP drain + barrier #1 (ends at the Pool
        # release EventSemaphore); drop: sem range-clear + barrier #2
        keep = []
        barrier_done = 0
        for ins in endbb.instructions:
            if barrier_done >= 1 and isinstance(
                ins, (mybir.InstDrain, mybir.InstISA)
            ):
                continue
            if barrier_done >= 1 and isinstance(ins, mybir.InstEventSemaphore):
                continue
            keep.append(ins)
            si = ins.sync_info
            if (
                isinstance(ins, mybir.InstEventSemaphore)
                and si
                and si.on_update
                and si.on_update[0].update_mode == "sem-add-imm"
                and si.on_update[0].update_value == 4
            ):
                barrier_done += 1
        endbb.instructions[:] = keep
    _fix_multi_waits(nc)
    _NC_CACHE[key] = nc
    return nc


last_results = None  # BassKernelResults of the most recent launch (for test.py)


def _pack_pkc(a, inner):
    """[KC*128, inner] -> [128, KC*inner] with row p holding [kc, inner]."""
    return (
        a.reshape(KC, 128, inner).transpose(1, 0, 2).reshape(128, KC * inner)
    )


def kernel(hidden_states, topk_weights, topk_ids, up_weight, down_weight):
    global last_results
    np_dt = np.float32 if MM_DTYPE == "f32r" else ml_dtypes.bfloat16
    hs = np.asarray(hidden_states, dtype=np.float32)
    twf = np.asarray(topk_weights, dtype=np.float32).ravel()
    ids = np.asarray(topk_ids).astype(np.int64).ravel()
    wu = np.asarray(up_weight, dtype=np.float32)
    wd = np.asarray(down_weight, dtype=np.float32)

    order = np.argsort(ids, kind="stable")
    counts = np.bincount(ids, minlength=E)
    starts = np.concatenate([[0], np.cumsum(counts)])
    hsT = np.ascontiguousarray(hs.T.astype(np_dt))  # [K, T]

    # slot assignment: 8 largest experts -> slot 0, 8 smallest -> slot 1;
    # per-slot capacity = that slot's max count (rounded to 2), so the NEFF
    # wastes no columns beyond imbalance between experts of the same slot
    by_size = np.argsort(-counts, kind="stable")
    slot_experts = [by_size[:NCORES], by_size[NCORES:]]  # [slot][core] -> e
    caps = []
    for s in range(EPC):
        m = int(counts[slot_experts[s]].max())
        caps.append(max(128, min(CAPMAX, (m + 1) // 2 * 2)))
    nds = [-(-c // 128) for c in caps]

    nc = _build(caps)

    wup_maps = [[], []]
    wdn_maps = [[], []]
    for s in range(EPC):
        for c in range(NCORES):
            e = int(slot_experts[s][c])
            wup_maps[s].append(
                np.ascontiguousarray(_pack_pkc(wu[e].T.astype(np_dt), H2))
            )
            # [H, K] -> [128, 2*K]: row p holds [hh, k] contiguously
            wdt = wd[e].T.astype(np_dt).reshape(2, 128, K)
            wdn_maps[s].append(
                np.ascontiguousarray(
                    wdt.transpose(1, 0, 2).reshape(128, 2 * K)
                )
            )

    out = np.zeros((T, K), np.float32)
    rounds = int(max(1, -(-int(counts.max()) // max(caps))))
    for r in range(rounds):
        in_maps = []
        toks = []  # per core: list of (slot, n, token_idx)
        for c in range(NCORES):
            wva = np.zeros((sum(nds), 128), np.float32)
            ct = []
            im = {"wv": None}
            for s in range(EPC):
                e = int(slot_experts[s][c])
                cap = caps[s]
                xTa = np.zeros((128, KC, cap), np_dt)
                lo = starts[e] + r * cap
                hi = min(starts[e + 1], lo + cap)
                seg = order[lo:hi] if hi > lo else np.empty(0, np.int64)
                n = len(seg)
                if n:
                    t = seg // TOPK
                    g = hsT[:, t].reshape(KC, 128, n)  # [kc, p, n]
                    xTa[:, :, :n] = g.transpose(1, 0, 2)
                    base = s * nds[0]
                    wva.reshape(-1)[base * 128 : base * 128 + n] = twf[seg]
                    ct.append((s, n, t))
                im[f"xT{s}"] = xTa.reshape(128, KC * cap)
                im[f"wup{s}"] = wup_maps[s][c]
                im[f"wdn{s}"] = wdn_maps[s][c]
            im["wv"] = np.ascontiguousarray(wva.T)
            toks.append(ct)
            in_maps.append(im)
        last_results = run_bass_kernel_spmd(
            nc, in_maps, core_ids=list(range(NCORES))
        )
        for c in range(NCORES):
            for s, n, t in toks[c]:
                yc = last_results.results[c][f"y{s}"][:n].astype(np.float32)
                np.add.at(out, t, yc)
    return out


# revision 16
# speedup vs baseline: 1.1562x; 1.1562x over previous
"""Fused MoE (top-2 routing) on 8 trn2 NeuronCores, expert-parallel.

Strategy: E=16 experts are sharded 2-per-core (two "slots"). The host groups
the T*TOPK (token, slot) pairs by expert (the all-to-all "dispatch"), assigns
the 8 largest experts to slot 0 and the 8 smallest to slot 1, and sizes the
two slot capacities (c0 >= c1) at runtime from the actual routing counts —
the kernel is rebuilt (and NEFF-cached) per capacity pair, so no compute or
DMA is spent on empty padding beyond round-to-2.

All matmul inputs and the y output travel as bf16 (fp32 accumulate in PSUM);
rel-err vs the f32 reference is ~5e-3, inside the 2e-2 gate, and DMA bytes
halve vs f32/f32r. Per core, per slot with capacity c:
  - xT_s  [128, 8*c]   gathered tokens, row p col kc*c+j = x[tok j, k=kc*128+p]
  - wup_s [128, 8*512] up_weight[e].T in the same [p, kc, f] layout
  - wdn_s [128, 2*1024] down_weight[e].T, row p col hh*1024+k
  - wv    [128, nd0+nd1] routing weight per pair, [p, token-chunk] layout
DMA here is descriptor-rate-bound (~80ns per ~2KB descriptor per SDMA
engine), so loads are merged into multi-kc tiles whose DRAM rows are single
1-8KB contiguous runs per partition (KSPLITS): slot 0 leads with a 1-kc tile
so the first matmul's dependency is small, slot 1 is one 8-kc tile per
tensor. Hard-won scheduling facts, all measured on HW traces:
  * The entry-block Drain waits for hoisted DMA transfers to COMPLETE, so
    hoisting N load triggers pre-barrier delays body start until those
    transfers land: NHOIST=2 (the two 1-kc slot-0 tiles) balances early
    body start (~9us) against having data ready.
  * Engines cannot execute before ~7us (IRAM preamble): a "warm-up during
    idle preamble" does not exist. Instead 8 wait-free dummy matmuls on a
    memset tile run at body start, back-to-back, so the PE HAM clock-gate
    (1.2GHz cold -> 2.4GHz after ~3.4us of sustained activity) flips right
    as the first real matmul issues; the real up-phase then runs warm
    (117ns per 275-row matmul instead of 227ns).
  * Stores are padded to all 128 partitions (rows past the slot capacity
    are junk the host ignores): partial-partition stores serialize onto 2
    of the 16 SDMA engines and once cost a 5.6us kernel tail.
  * Each dma_start trigger costs ~0.6us of descriptor generation on its
    engine's sequencer: loads ride the sync ring, stores the scalar ring,
    so the two trigger streams don't serialize against each other.

The device computes up.T = wupT.T @ xT (PSUM fp32), SwiGLU in the transposed
layout (no on-chip transposes), down = actT.T @ wdnT with the routing weight
applied on the PSUM->SBUF copy (ACT scales the first K-half under the second
half's matmuls; the faster DVE op gates the store), and writes y_s bf16.
Phases run up(0), down(0), up(1), down(1) so slot-0 stores stream mid-kernel.
The host scatter-adds y rows back to tokens in f32 (the "combine"). Tile's
redundant exit semaphore-clear/barrier is stripped; the remaining ~7us
post-barrier tail is the NEFF wrapper's fixed per-engine semaphore-reset
epilogue (~56 ops x 5 engines, not emitted by this kernel).
"""

import os

import ml_dtypes
import numpy as np

import concourse.bass as bass
import concourse.mybir as mybir
from concourse.bass_utils import run_bass_kernel_spmd
from concourse.tile import TileContext

T, K, H, E, TOPK = 4096, 1024, 256, 16, 2
H2 = 2 * H  # 512
NCORES = 8
EPC = E // NCORES  # experts (slots) per core = 2
KC = K // 128  # 8 contraction chunks
CAPMAX = 2048  # per-slot capacity bound (SBUF); above this, run more rounds

F32 = mybir.dt.float32
# matmul input dtype: "bf16" (half DMA bytes) or "f32r" (tf32-like, full bytes)
MM_DTYPE = os.environ.get("MOE_MM_DTYPE", "bf16")
Y_DTYPE = os.environ.get("MOE_Y_DTYPE", MM_DTYPE)
NHOIST = int(os.environ.get("MOE_NHOIST", "2"))
NWARM = int(os.environ.get("MOE_WARM", "8"))
WARM_HOIST = os.environ.get("MOE_WARM_HOIST", "0") == "1"
# kc-chunk grouping per slot: slot 0 leads with a 1-chunk tile so the first
# matmul's DMA dependency is ~0.3MB instead of ~1.1MB; the rest ride in
# wide tiles whose 3-9KB DRAM rows keep the SDMA descriptor rate efficient
KSPLITS = [[1, 3, 4], [8]]


def _up_tiles(c):
    """Split c token columns into PSUM-bank-sized (<=512) even chunks."""
    if c <= 512:
        return [c]
    u = (c + 3) // 4 * 2
    return [u, c - u]


def _fix_multi_waits(nc):
    """This walrus build accepts one sync-wait command per instruction (two
    for EventSemaphore); Tile's exit drain stacks every outstanding semaphore
    onto a single Drain. Move the excess waits onto no-ops inserted before
    the offending instruction on the same engine."""
    for f in nc.m.functions:
        for bb in f.blocks:
            i = 0
            while i < len(bb.instructions):
                ins = bb.instructions[i]
                si = ins.sync_info
                cap = 2 if isinstance(ins, mybir.InstEventSemaphore) else 1
                if si is not None and si.on_wait and len(si.on_wait) > cap:
                    waits = list(si.on_wait)
                    keep, extra = waits[:cap], waits[cap:]
                    nops = [
                        mybir.InstNoOp(
                            name=f"{ins.name}_waitfix{j}",
                            sync_info=mybir.SyncInfo(on_wait=[w], on_update=[]),
                            bass_nofuse=True,
                            engine=ins.engine,
                        )
                        for j, w in enumerate(extra)
                    ]
                    ins.sync_info = mybir.SyncInfo(
                        on_wait=keep, on_update=list(si.on_update)
                    )
                    bb.instructions[i:i] = nops
                    i += len(nops)
                i += 1


_NC_CACHE = {}


def _build(caps):
    key = (tuple(caps), MM_DTYPE, Y_DTYPE, NWARM, WARM_HOIST)
    if key in _NC_CACHE:
        return _NC_CACHE[key]
    DT = mybir.dt.float32r if MM_DTYPE == "f32r" else mybir.dt.bfloat16
    YDT = mybir.dt.float32 if Y_DTYPE == "f32r" else mybir.dt.bfloat16
    nds = [-(-c // 128) for c in caps]  # down-phase token tiles per slot
    nc = bass.Bass()
    xT = [
        nc.dram_tensor(f"xT{s}", [128, KC * caps[s]], DT, kind="ExternalInput")
        for s in range(EPC)
    ]
    wup = [
        nc.dram_tensor(f"wup{s}", [128, KC * H2], DT, kind="ExternalInput")
        for s in range(EPC)
    ]
    wdn = [
        nc.dram_tensor(f"wdn{s}", [128, 2 * K], DT, kind="ExternalInput")
        for s in range(EPC)
    ]
    wv = nc.dram_tensor("wv", [128, sum(nds)], F32, kind="ExternalInput")
    # y is padded to whole 128-token tiles: partial-partition store DMAs
    # serialize onto 2 of the 16 SDMA engines (measured: a 122-row store
    # took 5.6us as the kernel tail), full-128 stores fan out across all 16
    y = [
        nc.dram_tensor(f"y{s}", [nds[s] * 128, K], YDT, kind="ExternalOutput")
        for s in range(EPC)
    ]

    with TileContext(nc) as tc:
        with (
            tc.tile_pool(name="persist", bufs=1) as pp,
            tc.tile_pool(name="sil", bufs=4) as silp,
            tc.tile_pool(name="yout", bufs=6) as yp,
            tc.tile_pool(name="psum_up", bufs=2, space="PSUM") as psu,
            tc.tile_pool(name="psum_dn", bufs=2, space="PSUM") as psd,
        ):
            # PE warm-up: wait-free matmuls on never-written SBUF, hoisted
            # into the entry block so the HAM clock-gate sees ~3.4us of
            # sustained PE activity during the preamble and releases the
            # 1.2GHz->2.4GHz throttle before the first real matmul.
            if NWARM:
                wtile = pp.tile([128, 512], DT, tag="warm", name="wtile")
                nc.vector.memset(wtile[:], 0.0)
                pwarm = psd.tile([128, 512], F32, tag="dn0", name="pwarm")
                for _ in range(NWARM):
                    nc.tensor.matmul(
                        pwarm[:], wtile[:, :128], wtile[:], start=True, stop=True
                    )

            # one tile per (tensor, slot, half) so readers only gate on the
            # DMA that actually feeds them; each DMA's DRAM rows are one
            # contiguous 4-9KB run per partition
            xsb = [
                [
                    pp.tile(
                        [128, ks, caps[s]], DT, tag=f"x{s}_{h}", name=f"x{s}_{h}"
                    )
                    for h, ks in enumerate(KSPLITS[s])
                ]
                for s in range(EPC)
            ]
            wupsb = [
                [
                    pp.tile(
                        [128, ks, H2], DT, tag=f"wu{s}_{h}", name=f"wu{s}_{h}"
                    )
                    for h, ks in enumerate(KSPLITS[s])
                ]
                for s in range(EPC)
            ]
            wdnsb = [
                pp.tile([128, 2, K], DT, tag=f"wd{s}", name=f"wd{s}")
                for s in range(EPC)
            ]
            actsb = [
                [
                    pp.tile(
                        [128, caps[s]], DT, tag=f"a{s}_{hh}", name=f"a{s}_{hh}"
                    )
                    for hh in range(2)
                ]
                for s in range(EPC)
            ]
            wvsb = pp.tile([128, sum(nds)], F32)

            # all loads on the sync HWDGE queue, in consumption order
            kbase = [
                [sum(KSPLITS[s][:h]) for h in range(len(KSPLITS[s]) + 1)]
                for s in range(EPC)
            ]

            def load_wup(s, h):
                ks = KSPLITS[s][h]
                k0 = kbase[s][h]
                nc.sync.dma_start(
                    wupsb[s][h][:],
                    wup[s][
                        :, k0 * H2 : (k0 + ks) * H2
                    ].rearrange("p (kc f) -> p kc f", kc=ks),
                )

            def load_x(s, h):
                c = caps[s]
                ks = KSPLITS[s][h]
                k0 = kbase[s][h]
                nc.sync.dma_start(
                    xsb[s][h][:],
                    xT[s][
                        :, k0 * c : (k0 + ks) * c
                    ].rearrange("p (kc j) -> p kc j", kc=ks),
                )

            def load_wdn(s):
                nc.sync.dma_start(
                    wdnsb[s][:],
                    wdn[s][:, :].rearrange("p (hh k) -> p hh k", hh=2),
                )

            for h in range(len(KSPLITS[0])):
                load_wup(0, h)
                load_x(0, h)
            load_wdn(0)
            nc.sync.dma_start(wvsb[:], wv[:, :])
            for h in range(len(KSPLITS[1])):
                load_wup(1, h)
                load_x(1, h)
            load_wdn(1)

            def ktile(s, kc):
                h = next(
                    i for i in range(len(KSPLITS[s])) if kbase[s][i + 1] > kc
                )
                return h, kc - kbase[s][h]

            def up_phase(s):
                # up.T in PSUM: [feature-on-partition, token-free]. Features
                # hh*128..hh*128+127 (gate) pair with 256+hh*128.. (proj);
                # process one hh-half at a time so only two PSUM tags are
                # live and halves pipeline through 2 bufs each.
                c0 = 0
                for ulen in _up_tiles(caps[s]):
                    for hh in range(2):
                        pg = psu.tile([128, 512], F32, tag="upA", name="pg")[
                            :, :ulen
                        ]
                        pj = psu.tile([128, 512], F32, tag="upB", name="pj")[
                            :, :ulen
                        ]
                        for kc in range(KC):
                            h, ki = ktile(s, kc)
                            rhs = xsb[s][h][:, ki, c0 : c0 + ulen]
                            nc.tensor.matmul(
                                pg,
                                wupsb[s][h][
                                    :, ki, hh * 128 : (hh + 1) * 128
                                ],
                                rhs,
                                start=(kc == 0),
                                stop=(kc == KC - 1),
                            )
                            nc.tensor.matmul(
                                pj,
                                wupsb[s][h][
                                    :, ki, 256 + hh * 128 : 384 + hh * 128
                                ],
                                rhs,
                                start=(kc == 0),
                                stop=(kc == KC - 1),
                            )
                        sil = silp.tile([128, 512], F32, tag="sil")
                        nc.scalar.activation(
                            sil[:, :ulen], pg, mybir.ActivationFunctionType.Silu
                        )
                        nc.vector.tensor_tensor(
                            actsb[s][hh][:, c0 : c0 + ulen],
                            sil[:, :ulen],
                            pj,
                            mybir.AluOpType.mult,
                        )
                    c0 += ulen

            def down_phase(s):
                # down: [token-on-partition, k-free]; routing weight applied
                # on the PSUM->SBUF copy (split across DVE and ACT); stores
                # follow the loads on the sync queue (the only ring that
                # fans out across all 16 SDMA engines)
                base = s * nds[0]
                for td in range(nds[s]):
                    pt = min(128, caps[s] - td * 128)
                    ysb = yp.tile([128, K], YDT, tag="y", name="ysb")
                    col = base + td
                    wcol = wvsb[:pt, col : col + 1]
                    pys = [
                        psd.tile([128, 512], F32, tag=f"dn{nn}", name=f"dn{nn}")
                        for nn in range(2)
                    ]
                    # scale of the first half (ACT, slower) runs while the
                    # second half's matmuls stream; the last scale before
                    # the store is the faster DVE op, shortening the
                    # end-of-kernel chain
                    for nn in range(2):
                        for hh in range(2):
                            nc.tensor.matmul(
                                pys[nn][:pt],
                                actsb[s][hh][:, td * 128 : td * 128 + pt],
                                wdnsb[s][:, hh, nn * 512 : (nn + 1) * 512],
                                start=(hh == 0),
                                stop=(hh == 1),
                            )
                        if nn == 0:
                            nc.scalar.mul(ysb[:pt, 0:512], pys[0][:pt], wcol)
                    nc.vector.tensor_scalar_mul(
                        ysb[:pt, 512:1024], pys[1][:pt], wcol
                    )
                    # store all 128 partitions even when pt<128 (rows past
                    # pt are stale junk the host never reads): partial-
                    # partition stores serialize onto 2 SDMA engines. The
                    # trigger rides the scalar-engine HWDGE ring so the
                    # ~0.6us-per-trigger descriptor generation doesn't
                    # contend with the sync ring's load triggers.
                    r0 = td * 128
                    nc.scalar.dma_start(y[s][r0 : r0 + 128, :], ysb[:])

            up_phase(0)
            down_phase(0)
            up_phase(1)
            down_phase(1)

    # Hoist the leading wait-free sync-engine DMA triggers (the loads) into
    # the entry block, ahead of the Tile entry barrier: the transfers then
    # stream during the preamble (IRAM loads, const memsets, barrier)
    # instead of after it. HWDGE triggers retire at descriptor push, so the
    # preamble barrier's Drain does not stall on the in-flight transfers;
    # the body's existing semaphore waits gate consumers exactly as before.
    f0 = nc.m.functions[0]
    blocks = list(f0.blocks)
    main_bb, body_bb = blocks[0], blocks[1]
    hoist = []
    for ins in body_bb.instructions:
        if (
            isinstance(ins, mybir.InstDMACopy)
            and str(ins.engine) == "EngineType.SP"
            and not (ins.sync_info and ins.sync_info.on_wait)
        ):
            hoist.append(ins)
            if len(hoist) >= NHOIST:
                break
        elif isinstance(ins, mybir.InstDMACopy) and str(ins.engine) == (
            "EngineType.SP"
        ):
            break
    if hoist:
        names = {h.name for h in hoist}
        body_bb.instructions[:] = [
            i for i in body_bb.instructions if i.name not in names
        ]
        ip = 0
        for idx, ins in enumerate(main_bb.instructions):
            if str(ins.engine) == "EngineType.SP":
                si = ins.sync_info
                if si and (si.on_wait or si.on_update):
                    break
                ip = idx + 1
        main_bb.instructions[ip:ip] = hoist

    if NWARM and WARM_HOIST:
        # Move the warm-up memset (DVE, wait-free) and the warm-up
        # LDWEIGHTS/Matmults (PE; their only wait is the memset's semaphore,
        # which fires during the preamble) to the entry block, just before
        # each engine's barrier Drain: they then run during the preamble
        # while the PE would otherwise sit idle. Their semaphore updates
        # persist across the entry barrier — nothing in the entry block
        # resets semaphores.
        wh = []
        for ins in body_bb.instructions:
            if (
                isinstance(ins, mybir.InstMemset)
                and str(ins.engine) == "EngineType.DVE"
                and not (ins.sync_info and ins.sync_info.on_wait)
            ):
                wh.append(ins)
                break
        nmm = 0
        for ins in body_bb.instructions:
            if str(ins.engine) != "EngineType.PE":
                continue
            wh.append(ins)
            if isinstance(ins, mybir.InstMatmult):
                nmm += 1
                if nmm >= NWARM:
                    break
        if nmm == NWARM:
            names = {h.name for h in wh}
            body_bb.instructions[:] = [
                i for i in body_bb.instructions if i.name not in names
            ]
            for eng in ("EngineType.DVE", "EngineType.PE"):
                grp = [h for h in wh if str(h.engine) == eng]
                if not grp:
                    continue
                ip = next(
                    idx
                    for idx, ins in enumerate(main_bb.instructions)
                    if isinstance(ins, mybir.InstDrain)
                    and str(ins.engine) == eng
                )
                main_bb.instructions[ip:ip] = grp

    if True:  # drop Tile's exit sem-clear + second barrier (redundant with
        # the compiler's own per-engine semaphore-reset epilogue; verified
        # correct across repeated executions of the loaded NEFF)
        f = nc.m.functions[0]
        endbb = list(f.blocks)[-1]
        # keep: waitfix nops + SP drain + barrier #1 (ends at the Pool
        # release EventSemaphore); drop: sem range-clear + barrier #2
        keep = []
        barrier_done = 0
        for ins in endbb.instructions:
            if barrier_done >= 1 and isinstance(
                ins, (mybir.InstDrain, mybir.InstISA)
            ):
                continue
            if barrier_done >= 1 and isinstance(ins, mybir.InstEventSemaphore):
                continue
            keep.append(ins)
            si = ins.sync_info
            if (
                isinstance(ins, mybir.InstEventSemaphore)
                and si
                and si.on_update
                and si.on_update[0].update_mode == "sem-add-imm"
                and si.on_update[0].update_value == 4
            ):
                barrier_done += 1
        endbb.instructions[:] = keep
    _fix_multi_waits(nc)
    _NC_CACHE[key] = nc
    return nc


last_results = None  # BassKernelResults of the most recent launch (for test.py)


def _pack_pkc(a, inner):
    """[KC*128, inner] -> [128, KC*inner] with row p holding [kc, inner]."""
    return (
        a.reshape(KC, 128, inner).transpose(1, 0, 2).reshape(128, KC * inner)
    )


def kernel(hidden_states, topk_weights, topk_ids, up_weight, down_weight):
    global last_results
    np_dt = np.float32 if MM_DTYPE == "f32r" else ml_dtypes.bfloat16
    hs = np.asarray(hidden_states, dtype=np.float32)
    twf = np.asarray(topk_weights, dtype=np.float32).ravel()
    ids = np.asarray(topk_ids).astype(np.int64).ravel()
    wu = np.asarray(up_weight, dtype=np.float32)
    wd = np.asarray(down_weight, dtype=np.float32)

    order = np.argsort(ids, kind="stable")
    counts = np.bincount(ids, minlength=E)
    starts = np.concatenate([[0], np.cumsum(counts)])
    hsT = np.ascontiguousarray(hs.T.astype(np_dt))  # [K, T]

    # slot assignment: 8 largest experts -> slot 0, 8 smallest -> slot 1;
    # per-slot capacity = that slot's max count (rounded to 2), so the NEFF
    # wastes no columns beyond imbalance between experts of the same slot
    by_size = np.argsort(-counts, kind="stable")
    slot_experts = [by_size[:NCORES], by_size[NCORES:]]  # [slot][core] -> e
    caps = []
    for s in range(EPC):
        m = int(counts[slot_experts[s]].max())
        caps.append(max(128, min(CAPMAX, (m + 1) // 2 * 2)))
    nds = [-(-c // 128) for c in caps]

    nc = _build(caps)

    wup_maps = [[], []]
    wdn_maps = [[], []]
    for s in range(EPC):
        for c in range(NCORES):
            e = int(slot_experts[s][c])
            wup_maps[s].append(
                np.ascontiguousarray(_pack_pkc(wu[e].T.astype(np_dt), H2))
            )
            # [H, K] -> [128, 2*K]: row p holds [hh, k] contiguously
            wdt = wd[e].T.astype(np_dt).reshape(2, 128, K)
            wdn_maps[s].append(
                np.ascontiguousarray(
                    wdt.transpose(1, 0, 2).reshape(128, 2 * K)
                )
            )

    out = np.zeros((T, K), np.float32)
    rounds = int(max(1, -(-int(counts.max()) // max(caps))))
    for r in range(rounds):
        in_maps = []
        toks = []  # per core: list of (slot, n, token_idx)
        for c in range(NCORES):
            wva = np.zeros((sum(nds), 128), np.float32)
            ct = []
            im = {"wv": None}
            for s in range(EPC):
                e = int(slot_experts[s][c])
                cap = caps[s]
                xTa = np.zeros((128, KC, cap), np_dt)
                lo = starts[e] + r * cap
                hi = min(starts[e + 1], lo + cap)
                seg = order[lo:hi] if hi > lo else np.empty(0, np.int64)
                n = len(seg)
                if n:
                    t = seg // TOPK
                    g = hsT[:, t].reshape(KC, 128, n)  # [kc, p, n]
                    xTa[:, :, :n] = g.transpose(1, 0, 2)
                    base = s * nds[0]
                    wva.reshape(-1)[base * 128 : base * 128 + n] = twf[seg]
                    ct.append((s, n, t))
                im[f"xT{s}"] = xTa.reshape(128, KC * cap)
                im[f"wup{s}"] = wup_maps[s][c]
                im[f"wdn{s}"] = wdn_maps[s][c]
            im["wv"] = np.ascontiguousarray(wva.T)
            toks.append(ct)
            in_maps.append(im)
        last_results = run_bass_kernel_spmd(
            nc, in_maps, core_ids=list(range(NCORES))
        )
        for c in range(NCORES):
            for s, n, t in toks[c]:
                yc = last_results.results[c][f"y{s}"][:n].astype(np.float32)
                np.add.at(out, t, yc)
    return out


# revision 18
# speedup vs baseline: 1.1906x; 1.0298x over previous
"""Fused MoE (top-2 routing) on 8 trn2 NeuronCores, expert-parallel.

Strategy: E=16 experts are sharded 2-per-core (two "slots"). The host groups
the T*TOPK (token, slot) pairs by expert (the all-to-all "dispatch"), assigns
the 8 largest experts to slot 0 and the 8 smallest to slot 1, and sizes the
two slot capacities (c0 >= c1) at runtime from the actual routing counts —
the kernel is rebuilt (and NEFF-cached) per capacity pair, so no compute or
DMA is spent on empty padding beyond round-to-2.

All matmul inputs and the y output travel as bf16 (fp32 accumulate in PSUM);
rel-err vs the f32 reference is ~5e-3, inside the 2e-2 gate, and DMA bytes
halve vs f32/f32r. Per core, per slot with capacity c:
  - xT_s  [128, 8*c]   gathered tokens, row p col kc*c+j = x[tok j, k=kc*128+p]
  - wup_s [128, 8*512] up_weight[e].T in the same [p, kc, f] layout
  - wdn_s [128, 2*1024] down_weight[e].T, row p col hh*1024+k
  - wv    [128, nd0+nd1] routing weight per pair, [p, token-chunk] layout
DMA here is descriptor-rate-bound (~80ns per ~2KB descriptor per SDMA
engine), so loads are merged into multi-kc tiles whose DRAM rows are single
1-8KB contiguous runs per partition (KSPLITS): slot 0 leads with a 1-kc tile
so the first matmul's dependency is small, slot 1 is one 8-kc tile per
tensor. Hard-won scheduling facts, all measured on HW traces:
  * The entry-block Drain waits for hoisted DMA transfers to COMPLETE, so
    hoisting N load triggers pre-barrier delays body start until those
    transfers land: NHOIST=2 (the two 1-kc slot-0 tiles) balances early
    body start (~9us) against having data ready.
  * Engines cannot execute before ~7us (IRAM preamble): a "warm-up during
    idle preamble" does not exist. Instead 8 wait-free dummy matmuls on a
    memset tile run at body start, back-to-back, so the PE HAM clock-gate
    (1.2GHz cold -> 2.4GHz after ~3.4us of sustained activity) flips right
    as the first real matmul issues; the real up-phase then runs warm
    (117ns per 275-row matmul instead of 227ns).
  * Stores are padded to all 128 partitions (rows past the slot capacity
    are junk the host ignores): partial-partition stores serialize onto 2
    of the 16 SDMA engines and once cost a 5.6us kernel tail.
  * Each dma_start trigger costs ~0.6us of descriptor generation on its
    engine's sequencer: loads ride the sync ring, stores the scalar ring,
    so the two trigger streams don't serialize against each other.

The device computes up.T = wupT.T @ xT (PSUM fp32), SwiGLU in the transposed
layout (no on-chip transposes), down = actT.T @ wdnT with the routing weight
applied on the PSUM->SBUF copy (ACT scales the first K-half under the second
half's matmuls; the faster DVE op gates the store), and writes y_s bf16.
Phases run up(0), down(0), up(1), down(1) so slot-0 stores stream mid-kernel.
The host scatter-adds y rows back to tokens in f32 (the "combine"). Tile's
redundant exit semaphore-clear/barrier is stripped; the remaining ~7us
post-barrier tail is the NEFF wrapper's fixed per-engine semaphore-reset
epilogue (~56 ops x 5 engines, not emitted by this kernel).
"""

import os

import ml_dtypes
import numpy as np

import concourse.bass as bass
import concourse.mybir as mybir
from concourse.bass_utils import run_bass_kernel_spmd
from concourse.tile import TileContext

T, K, H, E, TOPK = 4096, 1024, 256, 16, 2
H2 = 2 * H  # 512
NCORES = 8
EPC = E // NCORES  # experts (slots) per core = 2
KC = K // 128  # 8 contraction chunks
CAPMAX = 2048  # per-slot capacity bound (SBUF); above this, run more rounds

F32 = mybir.dt.float32
# matmul input dtype: "bf16" (half DMA bytes) or "f32r" (tf32-like, full bytes)
MM_DTYPE = os.environ.get("MOE_MM_DTYPE", "bf16")
Y_DTYPE = os.environ.get("MOE_Y_DTYPE", MM_DTYPE)
NHOIST = int(os.environ.get("MOE_NHOIST", "2"))
NWARM = int(os.environ.get("MOE_WARM", "8"))
WARM_HOIST = os.environ.get("MOE_WARM_HOIST", "0") == "1"
# kc-chunk grouping per slot: slot 0 leads with a 1-chunk tile so the first
# matmul's DMA dependency is ~0.3MB instead of ~1.1MB; the rest ride in
# wide tiles whose 3-9KB DRAM rows keep the SDMA descriptor rate efficient
KSPLITS = [[1, 3, 4], [8]]


def _up_tiles(c):
    """Split c token columns into PSUM-bank-sized (<=512) even chunks."""
    if c <= 512:
        return [c]
    u = (c + 3) // 4 * 2
    return [u, c - u]


def _fix_multi_waits(nc):
    """This walrus build accepts one sync-wait command per instruction (two
    for EventSemaphore); Tile's exit drain stacks every outstanding semaphore
    onto a single Drain. Move the excess waits onto no-ops inserted before
    the offending instruction on the same engine."""
    for f in nc.m.functions:
        for bb in f.blocks:
            i = 0
            while i < len(bb.instructions):
                ins = bb.instructions[i]
                si = ins.sync_info
                cap = 2 if isinstance(ins, mybir.InstEventSemaphore) else 1
                if si is not None and si.on_wait and len(si.on_wait) > cap:
                    waits = list(si.on_wait)
                    keep, extra = waits[:cap], waits[cap:]
                    nops = [
                        mybir.InstNoOp(
                            name=f"{ins.name}_waitfix{j}",
                            sync_info=mybir.SyncInfo(on_wait=[w], on_update=[]),
                            bass_nofuse=True,
                            engine=ins.engine,
                        )
                        for j, w in enumerate(extra)
                    ]
                    ins.sync_info = mybir.SyncInfo(
                        on_wait=keep, on_update=list(si.on_update)
                    )
                    bb.instructions[i:i] = nops
                    i += len(nops)
                i += 1


_NC_CACHE = {}

# Declared kernel semaphore count (0 = stock range [walrus_max, 256)).
# Measured: shrinking the declared range changes NEITHER the gpsimd
# prologue (const memsets, not sem-clears) NOR the NEFF wrapper's
# per-engine semaphore-reset epilogue (~56 fixed resets x 5 engines after
# the exit barrier) — that ~9us tail is compiler-fixed. Left as an env
# knob only.
NSEMS = int(os.environ.get("MOE_NSEMS", "0"))


def _build(caps):
    key = (tuple(caps), MM_DTYPE, Y_DTYPE, NWARM, WARM_HOIST, NSEMS)
    if key in _NC_CACHE:
        return _NC_CACHE[key]
    DT = mybir.dt.float32r if MM_DTYPE == "f32r" else mybir.dt.bfloat16
    YDT = mybir.dt.float32 if Y_DTYPE == "f32r" else mybir.dt.bfloat16
    nds = [-(-c // 128) for c in caps]  # down-phase token tiles per slot
    orig_range = bass.get_kernel_semaphore_range
    if NSEMS:
        lo = orig_range().start
        bass.get_kernel_semaphore_range = lambda: range(
            lo, min(lo + NSEMS, 256)
        )
    try:
        nc = bass.Bass()
    finally:
        bass.get_kernel_semaphore_range = orig_range
    xT = [
        nc.dram_tensor(f"xT{s}", [128, KC * caps[s]], DT, kind="ExternalInput")
        for s in range(EPC)
    ]
    wup = [
        nc.dram_tensor(f"wup{s}", [128, KC * H2], DT, kind="ExternalInput")
        for s in range(EPC)
    ]
    wdn = [
        nc.dram_tensor(f"wdn{s}", [128, 2 * K], DT, kind="ExternalInput")
        for s in range(EPC)
    ]
    wv = nc.dram_tensor("wv", [128, sum(nds)], F32, kind="ExternalInput")
    # y is padded to whole 128-token tiles: partial-partition store DMAs
    # serialize onto 2 of the 16 SDMA engines (measured: a 122-row store
    # took 5.6us as the kernel tail), full-128 stores fan out across all 16
    y = [
        nc.dram_tensor(f"y{s}", [nds[s] * 128, K], YDT, kind="ExternalOutput")
        for s in range(EPC)
    ]

    with TileContext(nc) as tc:
        with (
            tc.tile_pool(name="persist", bufs=1) as pp,
            tc.tile_pool(name="sil", bufs=4) as silp,
            tc.tile_pool(name="yout", bufs=6) as yp,
            tc.tile_pool(name="psum_up", bufs=2, space="PSUM") as psu,
            tc.tile_pool(name="psum_dn", bufs=2, space="PSUM") as psd,
        ):
            # PE warm-up: wait-free matmuls on never-written SBUF, hoisted
            # into the entry block so the HAM clock-gate sees ~3.4us of
            # sustained PE activity during the preamble and releases the
            # 1.2GHz->2.4GHz throttle before the first real matmul.
            if NWARM:
                wtile = pp.tile([128, 512], DT, tag="warm", name="wtile")
                nc.vector.memset(wtile[:], 0.0)
                pwarm = psd.tile([128, 512], F32, tag="dn0", name="pwarm")
                for _ in range(NWARM):
                    nc.tensor.matmul(
                        pwarm[:], wtile[:, :128], wtile[:], start=True, stop=True
                    )

            # one tile per (tensor, slot, half) so readers only gate on the
            # DMA that actually feeds them; each DMA's DRAM rows are one
            # contiguous 4-9KB run per partition
            xsb = [
                [
                    pp.tile(
                        [128, ks, caps[s]], DT, tag=f"x{s}_{h}", name=f"x{s}_{h}"
                    )
                    for h, ks in enumerate(KSPLITS[s])
                ]
                for s in range(EPC)
            ]
            wupsb = [
                [
                    pp.tile(
                        [128, ks, H2], DT, tag=f"wu{s}_{h}", name=f"wu{s}_{h}"
                    )
                    for h, ks in enumerate(KSPLITS[s])
                ]
                for s in range(EPC)
            ]
            wdnsb = [
                pp.tile([128, 2, K], DT, tag=f"wd{s}", name=f"wd{s}")
                for s in range(EPC)
            ]
            actsb = [
                [
                    pp.tile(
                        [128, caps[s]], DT, tag=f"a{s}_{hh}", name=f"a{s}_{hh}"
                    )
                    for hh in range(2)
                ]
                for s in range(EPC)
            ]
            wvsb = pp.tile([128, sum(nds)], F32)

            # all loads on the sync HWDGE queue, in consumption order
            kbase = [
                [sum(KSPLITS[s][:h]) for h in range(len(KSPLITS[s]) + 1)]
                for s in range(EPC)
            ]

            def load_wup(s, h):
                ks = KSPLITS[s][h]
                k0 = kbase[s][h]
                nc.sync.dma_start(
                    wupsb[s][h][:],
                    wup[s][
                        :, k0 * H2 : (k0 + ks) * H2
                    ].rearrange("p (kc f) -> p kc f", kc=ks),
                )

            def load_x(s, h):
                c = caps[s]
                ks = KSPLITS[s][h]
                k0 = kbase[s][h]
                nc.sync.dma_start(
                    xsb[s][h][:],
                    xT[s][
                        :, k0 * c : (k0 + ks) * c
                    ].rearrange("p (kc j) -> p kc j", kc=ks),
                )

            def load_wdn(s):
                nc.sync.dma_start(
                    wdnsb[s][:],
                    wdn[s][:, :].rearrange("p (hh k) -> p hh k", hh=2),
                )

            for h in range(len(KSPLITS[0])):
                load_wup(0, h)
                load_x(0, h)
            load_wdn(0)
            nc.sync.dma_start(wvsb[:], wv[:, :])
            for h in range(len(KSPLITS[1])):
                load_wup(1, h)
                load_x(1, h)
            load_wdn(1)

            def ktile(s, kc):
                h = next(
                    i for i in range(len(KSPLITS[s])) if kbase[s][i + 1] > kc
                )
                return h, kc - kbase[s][h]

            def up_phase(s):
                # up.T in PSUM: [feature-on-partition, token-free]. Features
                # hh*128..hh*128+127 (gate) pair with 256+hh*128.. (proj);
                # process one hh-half at a time so only two PSUM tags are
                # live and halves pipeline through 2 bufs each.
                c0 = 0
                for ulen in _up_tiles(caps[s]):
                    for hh in range(2):
                        pg = psu.tile([128, 512], F32, tag="upA", name="pg")[
                            :, :ulen
                        ]
                        pj = psu.tile([128, 512], F32, tag="upB", name="pj")[
                            :, :ulen
                        ]
                        for kc in range(KC):
                            h, ki = ktile(s, kc)
                            rhs = xsb[s][h][:, ki, c0 : c0 + ulen]
                            nc.tensor.matmul(
                                pg,
                                wupsb[s][h][
                                    :, ki, hh * 128 : (hh + 1) * 128
                                ],
                                rhs,
                                start=(kc == 0),
                                stop=(kc == KC - 1),
                            )
                            nc.tensor.matmul(
                                pj,
                                wupsb[s][h][
                                    :, ki, 256 + hh * 128 : 384 + hh * 128
                                ],
                                rhs,
                                start=(kc == 0),
                                stop=(kc == KC - 1),
                            )
                        sil = silp.tile([128, 512], F32, tag="sil")
                        nc.scalar.activation(
                            sil[:, :ulen], pg, mybir.ActivationFunctionType.Silu
                        )
                        nc.vector.tensor_tensor(
                            actsb[s][hh][:, c0 : c0 + ulen],
                            sil[:, :ulen],
                            pj,
                            mybir.AluOpType.mult,
                        )
                    c0 += ulen

            def down_phase(s):
                # down: [token-on-partition, k-free]; routing weight applied
                # on the PSUM->SBUF copy (split across DVE and ACT); stores
                # follow the loads on the sync queue (the only ring that
                # fans out across all 16 SDMA engines)
                base = s * nds[0]
                for td in range(nds[s]):
                    pt = min(128, caps[s] - td * 128)
                    ysb = yp.tile([128, K], YDT, tag="y", name="ysb")
                    col = base + td
                    wcol = wvsb[:pt, col : col + 1]
                    pys = [
                        psd.tile([128, 512], F32, tag=f"dn{nn}", name=f"dn{nn}")
                        for nn in range(2)
                    ]
                    # scale of the first half (ACT, slower) runs while the
                    # second half's matmuls stream; the last scale before
                    # the store is the faster DVE op, shortening the
                    # end-of-kernel chain
                    for nn in range(2):
                        for hh in range(2):
                            nc.tensor.matmul(
                                pys[nn][:pt],
                                actsb[s][hh][:, td * 128 : td * 128 + pt],
                                wdnsb[s][:, hh, nn * 512 : (nn + 1) * 512],
                                start=(hh == 0),
                                stop=(hh == 1),
                            )
                        if nn == 0:
                            nc.scalar.mul(ysb[:pt, 0:512], pys[0][:pt], wcol)
                    nc.vector.tensor_scalar_mul(
                        ysb[:pt, 512:1024], pys[1][:pt], wcol
                    )
                    # store all 128 partitions even when pt<128 (rows past
                    # pt are stale junk the host never reads): partial-
                    # partition stores serialize onto 2 SDMA engines. The
                    # trigger rides the scalar-engine HWDGE ring so the
                    # ~0.6us-per-trigger descriptor generation doesn't
                    # contend with the sync ring's load triggers.
                    r0 = td * 128
                    nc.scalar.dma_start(y[s][r0 : r0 + 128, :], ysb[:])

            up_phase(0)
            down_phase(0)
            up_phase(1)
            down_phase(1)

    # Hoist the leading wait-free sync-engine DMA triggers (the loads) into
    # the entry block, ahead of the Tile entry barrier: the transfers then
    # stream during the preamble (IRAM loads, const memsets, barrier)
    # instead of after it. HWDGE triggers retire at descriptor push, so the
    # preamble barrier's Drain does not stall on the in-flight transfers;
    # the body's existing semaphore waits gate consumers exactly as before.
    f0 = nc.m.functions[0]
    blocks = list(f0.blocks)
    main_bb, body_bb = blocks[0], blocks[1]
    hoist = []
    for ins in body_bb.instructions:
        if (
            isinstance(ins, mybir.InstDMACopy)
            and str(ins.engine) == "EngineType.SP"
            and not (ins.sync_info and ins.sync_info.on_wait)
        ):
            hoist.append(ins)
            if len(hoist) >= NHOIST:
                break
        elif isinstance(ins, mybir.InstDMACopy) and str(ins.engine) == (
            "EngineType.SP"
        ):
            break
    if hoist:
        names = {h.name for h in hoist}
        body_bb.instructions[:] = [
            i for i in body_bb.instructions if i.name not in names
        ]
        ip = 0
        for idx, ins in enumerate(main_bb.instructions):
            if str(ins.engine) == "EngineType.SP":
                si = ins.sync_info
                if si and (si.on_wait or si.on_update):
                    break
                ip = idx + 1
        main_bb.instructions[ip:ip] = hoist

    if NWARM and WARM_HOIST:
        # Move the warm-up memset (DVE, wait-free) and the warm-up
        # LDWEIGHTS/Matmults (PE; their only wait is the memset's semaphore,
        # which fires during the preamble) to the entry block, just before
        # each engine's barrier Drain: they then run during the preamble
        # while the PE would otherwise sit idle. Their semaphore updates
        # persist across the entry barrier — nothing in the entry block
        # resets semaphores.
        wh = []
        for ins in body_bb.instructions:
            if (
                isinstance(ins, mybir.InstMemset)
                and str(ins.engine) == "EngineType.DVE"
                and not (ins.sync_info and ins.sync_info.on_wait)
            ):
                wh.append(ins)
                break
        nmm = 0
        for ins in body_bb.instructions:
            if str(ins.engine) != "EngineType.PE":
                continue
            wh.append(ins)
            if isinstance(ins, mybir.InstMatmult):
                nmm += 1
                if nmm >= NWARM:
                    break
        if nmm == NWARM:
            names = {h.name for h in wh}
            body_bb.instructions[:] = [
                i for i in body_bb.instructions if i.name not in names
            ]
            for eng in ("EngineType.DVE", "EngineType.PE"):
                grp = [h for h in wh if str(h.engine) == eng]
                if not grp:
                    continue
                ip = next(
                    idx
                    for idx, ins in enumerate(main_bb.instructions)
                    if isinstance(ins, mybir.InstDrain)
                    and str(ins.engine) == eng
                )
                main_bb.instructions[ip:ip] = grp

    if True:  # drop Tile's exit sem-clear + second barrier (redundant with
        # the compiler's own per-engine semaphore-reset epilogue; verified
        # correct across repeated executions of the loaded NEFF)
        f = nc.m.functions[0]
        endbb = list(f.blocks)[-1]
        # keep: waitfix nops + SP drain + barrier #1 (ends at the Pool
        # release EventSemaphore); drop: sem range-clear + barrier #2
        keep = []
        barrier_done = 0
        for ins in endbb.instructions:
            if barrier_done >= 1 and isinstance(
                ins, (mybir.InstDrain, mybir.InstISA)
            ):
                continue
            if barrier_done >= 1 and isinstance(ins, mybir.InstEventSemaphore):
                continue
            keep.append(ins)
            si = ins.sync_info
            if (
                isinstance(ins, mybir.InstEventSemaphore)
                and si
                and si.on_update
                and si.on_update[0].update_mode == "sem-add-imm"
                and si.on_update[0].update_value == 4
            ):
                barrier_done += 1
        endbb.instructions[:] = keep
    _fix_multi_waits(nc)
    _NC_CACHE[key] = nc
    return nc


last_results = None  # BassKernelResults of the most recent launch (for test.py)


def _pack_pkc(a, inner):
    """[KC*128, inner] -> [128, KC*inner] with row p holding [kc, inner]."""
    return (
        a.reshape(KC, 128, inner).transpose(1, 0, 2).reshape(128, KC * inner)
    )


def kernel(hidden_states, topk_weights, topk_ids, up_weight, down_weight):
    global last_results
    np_dt = np.float32 if MM_DTYPE == "f32r" else ml_dtypes.bfloat16
    hs = np.asarray(hidden_states, dtype=np.float32)
    twf = np.asarray(topk_weights, dtype=np.float32).ravel()
    ids = np.asarray(topk_ids).astype(np.int64).ravel()
    wu = np.asarray(up_weight, dtype=np.float32)
    wd = np.asarray(down_weight, dtype=np.float32)

    order = np.argsort(ids, kind="stable")
    counts = np.bincount(ids, minlength=E)
    starts = np.concatenate([[0], np.cumsum(counts)])
    hsT = np.ascontiguousarray(hs.T.astype(np_dt))  # [K, T]

    # slot assignment: 8 largest experts -> slot 0, 8 smallest -> slot 1;
    # per-slot capacity = that slot's max count (rounded to 2), so the NEFF
    # wastes no columns beyond imbalance between experts of the same slot
    by_size = np.argsort(-counts, kind="stable")
    slot_experts = [by_size[:NCORES], by_size[NCORES:]]  # [slot][core] -> e
    caps = []
    for s in range(EPC):
        m = int(counts[slot_experts[s]].max())
        caps.append(max(128, min(CAPMAX, (m + 1) // 2 * 2)))
    nds = [-(-c // 128) for c in caps]

    nc = _build(caps)

    wup_maps = [[], []]
    wdn_maps = [[], []]
    for s in range(EPC):
        for c in range(NCORES):
            e = int(slot_experts[s][c])
            wup_maps[s].append(
                np.ascontiguousarray(_pack_pkc(wu[e].T.astype(np_dt), H2))
            )
            # [H, K] -> [128, 2*K]: row p holds [hh, k] contiguously
            wdt = wd[e].T.astype(np_dt).reshape(2, 128, K)
            wdn_maps[s].append(
                np.ascontiguousarray(
                    wdt.transpose(1, 0, 2).reshape(128, 2 * K)
                )
            )

    out = np.zeros((T, K), np.float32)
    rounds = int(max(1, -(-int(counts.max()) // max(caps))))
    for r in range(rounds):
        in_maps = []
        toks = []  # per core: list of (slot, n, token_idx)
        for c in range(NCORES):
            wva = np.zeros((sum(nds), 128), np.float32)
            ct = []
            im = {"wv": None}
            for s in range(EPC):
                e = int(slot_experts[s][c])
                cap = caps[s]
                xTa = np.zeros((128, KC, cap), np_dt)
                lo = starts[e] + r * cap
                hi = min(starts[e + 1], lo + cap)
                seg = order[lo:hi] if hi > lo else np.empty(0, np.int64)
                n = len(seg)
                if n:
                    t = seg // TOPK
                    g = hsT[:, t].reshape(KC, 128, n)  # [kc, p, n]
                    xTa[:, :, :n] = g.transpose(1, 0, 2)
                    base = s * nds[0]
                    wva.reshape(-1)[base * 128 : base * 128 + n] = twf[seg]
                    ct.append((s, n, t))
                im[f"xT{s}"] = xTa.reshape(128, KC * cap)
                im[f"wup{s}"] = wup_maps[s][c]
                im[f"wdn{s}"] = wdn_maps[s][c]
            im["wv"] = np.ascontiguousarray(wva.T)
            toks.append(ct)
            in_maps.append(im)
        last_results = run_bass_kernel_spmd(
            nc, in_maps, core_ids=list(range(NCORES))
        )
        for c in range(NCORES):
            for s, n, t in toks[c]:
                yc = last_results.results[c][f"y{s}"][:n].astype(np.float32)
                np.add.at(out, t, yc)
    return out
